# revision 76
# baseline (speedup 1.0000x reference)
"""Dilated multi-head attention (nn_DilatedMHA) on 8 trn2 NeuronCores.

Math (reference restructured):
  qkv = x @ Wqkv.T                      [b, n, 3, h, d]   b=2, n=8192, h=12, d=64
  Position p attends within its mod-2048 class {g, g+2048, g+4096, g+6144}
  (g = p % 2048, slot j = p // 2048):  r=1 full 4x4 softmax, r=2 parity-pair
  2x2 (g even), r=4 adds v (g % 4 == 0).  out is normalized by the channel
  sum over the whole sequence per (b, h*d), then projected by Wout.

Sharding: core c <- batch c//4, groups [(c%4)*512, +512), 4 blocks of 128.

Key restructure vs the 250us baseline: the near-cancelling channel sums
s_c = sum_n o[n,c] (whose tiny channels amplify v-noise ~1e4x) are computed
NOT from the mixed o but analytically:
  s_c = sum_m (sum_n w[n,m]) v[m,c] = sum_k cx[h,k] Wv[(h,d),k]
where cx[h,:] = sum_m c^h_m x[m,:] are per-head weighted column sums of x,
computed exactly (fp32 row-major x) with tiny PE matmuls, then pushed
through fp32 Wv (72 1-row matmuls).  This decouples s from the V
projection, so V drops from a 16.5 cyc/row 5-term split scheme to a single
fp16 pass, and the attention mix + out-projection run in fp16 (their error
is not amplified).  Q,K keep the fp16+two-fp8-DoubleRow scheme (weight
errors do feed s through c).  Emulated end-to-end error 8.7e-3 (limit
2e-2); the same emulator put the old baseline at 6.3e-3 which measured
3.03e-3 on HW.

Engines (V1 cost model: Pool TT [128,768]=640ns with no gpsimd penalty,
DVE fp32 460 / broadcast-operand 860, reduces DVE-only at 860, DMAs
serialize per ISSUING engine): score muls split 12 Pool / 4 DVE, reduces
(with a half-fold first: 260+460 < 860) + softmax on DVE, fp16 mix muls
on Pool + adds on DVE, evacuations + exp + xr/w8/wo DMAs on ACT,
wl8 DMA on Pool.  Per-block emission is software-pipelined
(projections(blk+1) before attention(blk)); blocks 2 and 3 run in
head-half windows with their score chains interleaved into block 3's QK
windows, because the DVE-only reduce chain after the last K tiles is the
tail bottleneck.  The collective is gated by draining all
previously-emitted work and blocks later-emitted work until it completes
(framework cross-engine barriers), so: everything s-critical is emitted
before it, and the mixes/transposes of blocks 2-3 (needed only by the
last half of phase B) are emitted after it, along with the in-place
ACT r-fold of fp16 wo.

Late refinements: score pairs ordered by chunk readiness (max(j,jp));
block-0 weight/x streams split across the SP/ACT/Pool DMA queues
(parallel in the cost model); the 4-core s-sum as one X-axis reduce;
~11 throwaway matmuls at the collective's exit warm the PE p-state so
phase B runs at full clock; the last y row-chunk DMAs per-half.

Remaining known costs: the 15.3us collective constant plus ~11us of
DMA-init/semaphore glue around it (a remote_dma s-exchange would cut
~13us but has no Tile-framework integration), and ~6us of DVE-only
score-reduce chain after the last K tiles.

Final round: PE warm-up emitted immediately after the collective and the
late-mix muls kept off DVE so the s_tot reduce + reciprocal + ACT r-fold
gate phase B with minimal glue; block-0 V-tile evacuations go through the
(early-idle) DVE because the ACT queue streams weights at t=0 and would
hold PSUM hostage; the last row-chunk's y DMAs issue from ACT.

Measured: 198309 ns cost-model device time (baseline 250071, -20.7%),
HW rel err 2.14e-3 abs-max-relative (limit 2e-2, baseline 3.03e-3).
"""

import sys

if "/opt/trn_rl_repo" not in sys.path:
    sys.path.insert(0, "/opt/trn_rl_repo")

import numpy as np

EMBED = 768
HEADS = 12
HD = 64
B = 2
N = 8192
NCORES = 8
GPC = 512           # groups per core
NBLK = 4            # blocks of 128 groups per core
KO = 6              # embed // 128

S15 = 32768.0       # PSUM scale (2^15)

# score-mul engine split: first MUL_POOL of the 16 (j,jp) pairs go to Pool
# (V1 cost model: Pool TT [128,768]=640ns vs DVE fp32 460ns; DVE carries the
# reduces (DVE-only) and is the tail bottleneck, so Pool takes most muls)
MUL_POOL = 13

_COMPILED = {}


def _build_program():
    import concourse.mybir as mybir
    import concourse.tile as tile
    from concourse import bacc

    F32 = mybir.dt.float32
    F16 = mybir.dt.float16
    F8 = mybir.dt.float8e4
    AX = mybir.AxisListType
    OP = mybir.AluOpType
    ACTF = mybir.ActivationFunctionType
    DR = mybir.MatmulPerfMode.DoubleRow

    nc = bacc.Bacc("TRN2", target_bir_lowering=False, debug=False, num_devices=NCORES)

    # --- DRAM I/O ---------------------------------------------------------
    xch_d = nc.dram_tensor("xch", [NBLK, 128, 4, KO, 128], F16, kind="ExternalInput")
    x8_d = nc.dram_tensor("x8", [NBLK, 128, 4, KO, 128], F8, kind="ExternalInput")
    xl8_d = nc.dram_tensor("xl8", [NBLK, 128, 4, KO, 128], F8, kind="ExternalInput")
    xr_d = nc.dram_tensor("xr", [NBLK, 128, 4, EMBED], F32, kind="ExternalInput")
    w16_d = nc.dram_tensor("w16", [128, KO, 2 * EMBED], F16, kind="ExternalInput")
    w8_d = nc.dram_tensor("w8", [128, KO, 2 * EMBED], F8, kind="ExternalInput")
    wl8_d = nc.dram_tensor("wl8", [128, KO, 2 * EMBED], F8, kind="ExternalInput")
    v16_d = nc.dram_tensor("v16", [128, KO, EMBED], F16, kind="ExternalInput")
    wvT_d = nc.dram_tensor("wvT", [128, KO, EMBED], F32, kind="ExternalInput")
    wo_d = nc.dram_tensor("wo", [128, KO, EMBED], F16, kind="ExternalInput")
    m2_d = nc.dram_tensor("m2", [128, 1], F32, kind="ExternalInput")
    m4_d = nc.dram_tensor("m4", [128, 1], F32, kind="ExternalInput")
    y_d = nc.dram_tensor("y", [4 * GPC, EMBED], F32, kind="ExternalOutput")

    with tile.TileContext(nc) as tc:
        with (
            tc.tile_pool(name="const", bufs=1) as constp,
            tc.tile_pool(name="oT", bufs=16) as oTp,
            tc.tile_pool(name="mm", bufs=6, space="PSUM") as mmp,
            tc.tile_pool(name="cxp", bufs=1, space="PSUM") as cxpp,
            tc.tile_pool(name="sp", bufs=1, space="PSUM") as spp,
            tc.tile_pool(name="dram", bufs=2, space="DRAM") as dramp,
        ):
            # --- long-lived SBUF -----------------------------------------
            w16_sb = constp.tile([128, KO, 2 * EMBED], F16)
            w8_sb = constp.tile([128, KO, 2 * EMBED], F8)
            wl8_sb = constp.tile([128, KO, 2 * EMBED], F8)
            v16_sb = constp.tile([128, KO, EMBED], F16)
            wvT_sb = constp.tile([128, KO, EMBED], F32)
            m2_sb = constp.tile([128, 1], F32)
            m4_sb = constp.tile([128, 1], F32)
            cx_acc = constp.tile([128, KO, HEADS], F32)
            nc.vector.memset(cx_acc[:], 0.0)
            wo_sb = constp.tile([128, KO, EMBED], F16)
            s_acc = constp.tile([128, KO], F32)

            oT_tiles = {}   # (blk, j) -> [128, KO, 128] f16

            with (
                tc.tile_pool(name="xc", bufs=4) as xcp,
                tc.tile_pool(name="xr", bufs=2) as xrp,
                tc.tile_pool(name="qkv", bufs=2) as qkvp,
                tc.tile_pool(name="kpool", bufs=1) as kp,
                tc.tile_pool(name="vpool", bufs=2) as vp,
                tc.tile_pool(name="att", bufs=1) as attp,
                tc.tile_pool(name="prod", bufs=2) as prodp,
                tc.tile_pool(name="o16", bufs=1) as o16p,
            ):
                # ---------- loads ----------------------------------------
                def load_chunk(blk, j):
                    xh_sb = xcp.tile([128, KO, 128], F16, tag="xch")
                    nc.sync.dma_start(xh_sb[:], xch_d[blk, :, j])
                    x8_sb = xcp.tile([128, KO, 128], F8, tag="x8")
                    nc.sync.dma_start(x8_sb[:], x8_d[blk, :, j])
                    xl8_sb = xcp.tile([128, KO, 128], F8, tag="xl8")
                    nc.sync.dma_start(xl8_sb[:], xl8_d[blk, :, j])
                    return xh_sb, x8_sb, xl8_sb

                def load_xr(blk):
                    xr_sb = xrp.tile([128, 4, EMBED], F32, tag="xr")
                    nc.scalar.dma_start(xr_sb[:], xr_d[blk])
                    return xr_sb

                # ---------- PE tiles -------------------------------------
                def qk_tile(xs, n, dest, j):
                    """QK output tile n (384 cols): fp16 main + fp8 DR corr."""
                    xh_sb, x8_sb, xl8_sb = xs
                    sl = slice(n * 384, (n + 1) * 384)
                    ps = mmp.tile([128, 384], F32, tag="mm")
                    for ko in range(KO):
                        nc.tensor.matmul(
                            ps[:], lhsT=xh_sb[:, ko, :], rhs=w16_sb[:, ko, sl],
                            start=(ko == 0), stop=False,
                        )
                    for t in range(3):
                        nc.tensor.matmul(
                            ps[:], lhsT=xl8_sb[:, 2 * t:2 * t + 2, :],
                            rhs=w8_sb[:, 2 * t:2 * t + 2, sl],
                            start=False, stop=False, perf_mode=DR,
                        )
                    for t in range(3):
                        nc.tensor.matmul(
                            ps[:], lhsT=x8_sb[:, 2 * t:2 * t + 2, :],
                            rhs=wl8_sb[:, 2 * t:2 * t + 2, sl],
                            start=False, stop=(t == 2), perf_mode=DR,
                        )
                    nc.scalar.activation(
                        dest[:, j, (n % 2) * 384:(n % 2 + 1) * 384], ps[:],
                        ACTF.Copy, scale=1.0 / S15,
                    )

                def v_tile(xs, half, dest, j, evac_dve=False):
                    """V tile: single fp16 pass (s no longer depends on v).
                    evac_dve: block 0 evacuates via DVE -- the ACT queue is
                    busy streaming weights there and would otherwise hold
                    PSUM hostage."""
                    xh_sb = xs[0]
                    sl = slice(half * 384, (half + 1) * 384)
                    ps = mmp.tile([128, 384], F32, tag="mm")
                    for ko in range(KO):
                        nc.tensor.matmul(
                            ps[:], lhsT=xh_sb[:, ko, :], rhs=v16_sb[:, ko, sl],
                            start=(ko == 0), stop=(ko == KO - 1),
                        )
                    if evac_dve:
                        nc.vector.tensor_scalar_mul(
                            dest[:, j, sl], ps[:], 1.0 / S15
                        )
                    else:
                        nc.scalar.activation(
                            dest[:, j, sl], ps[:], ACTF.Copy, scale=1.0 / S15,
                        )

                # ---------- attention ------------------------------------
                def softmax_weights(E, Wt, hs):
                    """Wt[:, j, jp, hs] from exp'd scores E (layout j,jp,h)."""
                    nh = hs.stop - hs.start
                    Z1 = attp.tile([128, 4, HEADS], F32, tag="Z1")
                    nc.vector.tensor_add(
                        Z1[:, :, hs], E[:, :, 0, hs], E[:, :, 1, hs]
                    )
                    nc.vector.tensor_add(
                        Z1[:, :, hs], Z1[:, :, hs], E[:, :, 2, hs]
                    )
                    nc.vector.tensor_add(
                        Z1[:, :, hs], Z1[:, :, hs], E[:, :, 3, hs]
                    )
                    R1 = attp.tile([128, 4, HEADS], F32, tag="R1")
                    nc.vector.reciprocal(R1[:, :, hs], Z1[:, :, hs])
                    W1 = attp.tile([128, 4, 4, HEADS], F32, tag="W1")
                    nc.vector.tensor_mul(
                        W1[:, :, :, hs],
                        E[:, :, :, hs],
                        R1[:, :, None, hs].to_broadcast((128, 4, 4, nh)),
                    )
                    # r2 parity pairs
                    Z2 = attp.tile([128, 4, HEADS], F32, tag="Z2")
                    for j in range(4):
                        par = j % 2
                        nc.vector.tensor_add(
                            Z2[:, j, hs], E[:, j, par, hs], E[:, j, par + 2, hs]
                        )
                    R2 = attp.tile([128, 4, HEADS], F32, tag="R2")
                    nc.vector.reciprocal(R2[:, :, hs], Z2[:, :, hs])
                    W2 = attp.tile([128, 4, 4, HEADS], F32, tag="W2")
                    nc.vector.memset(W2[:, :, :, hs], 0.0)
                    for par in (0, 1):
                        nc.vector.tensor_mul(
                            W2[:, par::2, par::2, hs],
                            E[:, par::2, par::2, hs],
                            R2[:, par::2, None, hs].to_broadcast((128, 2, 2, nh)),
                        )
                    nc.vector.scalar_tensor_tensor(
                        Wt[:, :, :, hs], W2[:, :, :, hs], m2_sb[:, 0:1],
                        W1[:, :, :, hs], OP.mult, OP.add,
                    )
                    for j in range(4):
                        nc.vector.tensor_scalar_add(
                            Wt[:, j, j, hs], Wt[:, j, j, hs], m4_sb[:, 0:1]
                        )

                def scores_part(ctx, hs, mul_pool=MUL_POOL):
                    """Scores, softmax, c-sums, Wt16 for head range hs."""
                    Q4, K4, V16b, xr_sb, S, E, Wt, Wt16, c, o16 = ctx
                    nh = hs.stop - hs.start
                    # scores: per-pair mul (Pool/DVE split) + DVE reduce,
                    # ordered by chunk readiness (pair needs Q[j] and K[jp])
                    pairs = sorted(
                        ((j, jp) for j in range(4) for jp in range(4)),
                        key=lambda t: max(t),
                    )
                    for i, (j, jp) in enumerate(pairs):
                        if i < mul_pool:
                            eng, ptag = nc.gpsimd, "prodg"
                        else:
                            eng, ptag = nc.vector, "prod"
                        pr = prodp.tile([128, HEADS, HD], F32, tag=ptag)
                        eng.tensor_mul(
                            pr[:, :nh], Q4[:, j, hs], K4[:, jp, hs]
                        )
                        # fold halves first: DVE reduce cost is ap-size bound,
                        # 260+460 beats a single 860ns 768-wide reduce
                        nc.vector.tensor_add(
                            pr[:, :nh, 0:HD // 2], pr[:, :nh, 0:HD // 2],
                            pr[:, :nh, HD // 2:HD]
                        )
                        nc.vector.reduce_sum(
                            S[:, j, jp, hs], pr[:, :nh, 0:HD // 2], axis=AX.X
                        )
                    nc.scalar.activation(
                        E[:, :, :, hs], S[:, :, :, hs], ACTF.Exp, scale=0.125
                    )
                    softmax_weights(E, Wt, hs)
                    # column sums c[:, jp, h] = sum_j Wt
                    nc.vector.tensor_add(
                        c[:, :, hs], Wt[:, 0, :, hs], Wt[:, 1, :, hs]
                    )
                    nc.vector.tensor_add(
                        c[:, :, hs], c[:, :, hs], Wt[:, 2, :, hs]
                    )
                    nc.vector.tensor_add(
                        c[:, :, hs], c[:, :, hs], Wt[:, 3, :, hs]
                    )
                    nc.scalar.copy(Wt16[:, :, :, hs], Wt[:, :, :, hs])

                def mix_part(ctx, hs, split=False):
                    """fp16 AV mix for head range hs.

                    split=False (mid-phase blocks): everything on Pool -- DVE
                    is the saturated tail engine.  split=True (post-collective
                    bundle): muls alternate Pool/DVE, adds on DVE."""
                    Q4, K4, V16b, xr_sb, S, E, Wt, Wt16, c, o16 = ctx
                    nh = hs.stop - hs.start
                    o4 = o16[:].rearrange("p j (h d) -> p j h d", d=HD)
                    for j in range(4):
                        oj = o4[:, j, hs]
                        for jp in range(4):
                            meng = nc.gpsimd
                            if split and jp % 2 == 1:
                                meng = nc.vector
                            aeng = nc.vector
                            wb = Wt16[:, j, jp, hs, None].to_broadcast(
                                (128, nh, HD)
                            )
                            if jp == 0:
                                meng.tensor_mul(oj, V16b[:, jp, hs], wb)
                            else:
                                pr = prodp.tile([128, HEADS, HD], F16, tag="pr16")
                                meng.tensor_mul(
                                    pr[:, :nh], V16b[:, jp, hs], wb
                                )
                                aeng.tensor_add(oj, oj, pr[:, :nh])

                def cx_part(ctx):
                    Q4, K4, V16b, xr_sb, S, E, Wt, Wt16, c, o16 = ctx
                    # cx: cxT[k, h] += sum_g xr[g, j, k] * c[g, j, h]
                    cxps = cxpp.tile([128, KO, HEADS], F32, tag="cx")
                    xr4 = xr_sb[:].rearrange("p j (ko k) -> p j ko k", k=128)
                    for ko in range(KO):
                        for j in range(4):
                            nc.tensor.matmul(
                                cxps[:, ko, :], lhsT=xr4[:, j, ko, :],
                                rhs=c[:, j, :],
                                start=(j == 0), stop=(j == 3),
                            )
                    nc.vector.tensor_add(cx_acc[:], cx_acc[:], cxps[:])

                def transpose_part(blk, ctx):
                    o16 = ctx[9]
                    # oT transposes (batched 6 xbar tiles per call)
                    for j in range(4):
                        oT = oTp.tile([128, KO, 128], F16, tag="oT")
                        nc.sync.dma_start_transpose(
                            oT[:, :, :], o16[:, j, :]
                        )
                        oT_tiles[(blk, j)] = oT

                def attention(blk, ctx, hs):
                    scores_part(ctx, hs)
                    mix_part(ctx, hs)
                    cx_part(ctx)
                    transpose_part(blk, ctx)

                def make_ctx(blk):
                    Qb = qkvp.tile([128, 4, EMBED], F32, tag="qb")
                    Kb = kp.tile([128, 4, EMBED], F32, tag="kb")
                    Vb = vp.tile([128, 4, EMBED], F16, tag="vb")
                    Q4 = Qb[:].rearrange("p j (h d) -> p j h d", d=HD)
                    K4 = Kb[:].rearrange("p j (h d) -> p j h d", d=HD)
                    V4 = Vb[:].rearrange("p j (h d) -> p j h d", d=HD)
                    xr_sb = load_xr(blk)
                    S = attp.tile([128, 4, 4, HEADS], F32, tag="S")
                    E = attp.tile([128, 4, 4, HEADS], F32, tag="E")
                    Wt = attp.tile([128, 4, 4, HEADS], F32, tag="Wt")
                    Wt16 = attp.tile([128, 4, 4, HEADS], F16, tag="Wt16")
                    c = attp.tile([128, 4, HEADS], F32, tag="c")
                    o16 = o16p.tile([128, 4, EMBED], F16, tag="o16")
                    return (Qb, Kb, Vb), (Q4, K4, V4, xr_sb, S, E, Wt, Wt16,
                                          c, o16)

                def projections(blk, dests, xss, windows=((0, 1), (2, 3))):
                    """QK tiles (grouped by window) then V tiles."""
                    Qb, Kb, Vb = dests
                    for win in windows:
                        for j in range(4):
                            for n in win:
                                qk_tile(xss[j], n, Qb if n < 2 else Kb, j)
                    for j in range(4):
                        for half in range(2):
                            v_tile(xss[j], half, Vb, j)

                # ---------- main schedule --------------------------------
                # block-0 loads stream on three parallel DMA queues (the
                # cost model serializes DMAs per ISSUING engine):
                #   SP:  v16 (interleaved with xh chunks -- the V tiles are
                #        the earliest PE work), then w16[0:2]
                #   ACT: w8, w16[2:4], x8 chunks
                #   Pool: wl8, w16[4:6], xl8 chunks
                xss = {0: {}}
                xh_t = {}
                nc.sync.dma_start(v16_sb[:, 0, :], v16_d[:, 0, :])
                for j in range(4):
                    xh_sb = xcp.tile([128, KO, 128], F16, tag="xch")
                    nc.sync.dma_start(xh_sb[:], xch_d[0, :, j])
                    xh_t[j] = xh_sb
                    if j + 1 < KO:
                        nc.sync.dma_start(v16_sb[:, j + 1, :], v16_d[:, j + 1, :])
                for ko in range(4, KO):
                    nc.sync.dma_start(v16_sb[:, ko, :], v16_d[:, ko, :])
                nc.sync.dma_start(w16_sb[:, 0, :], w16_d[:, 0, :])
                nc.sync.dma_start(w16_sb[:, 1, :], w16_d[:, 1, :])
                nc.sync.dma_start(m2_sb[:], m2_d[:])
                nc.sync.dma_start(m4_sb[:], m4_d[:])
                for ko in range(KO):
                    nc.scalar.dma_start(w8_sb[:, ko, :], w8_d[:, ko, :])
                nc.scalar.dma_start(w16_sb[:, 2, :], w16_d[:, 2, :])
                nc.scalar.dma_start(w16_sb[:, 3, :], w16_d[:, 3, :])
                for ko in range(KO):
                    nc.gpsimd.dma_start(wl8_sb[:, ko, :], wl8_d[:, ko, :])
                nc.gpsimd.dma_start(w16_sb[:, 4, :], w16_d[:, 4, :])
                nc.gpsimd.dma_start(w16_sb[:, 5, :], w16_d[:, 5, :])
                x8_t = {}
                for j in range(4):
                    x8_sb = xcp.tile([128, KO, 128], F8, tag="x8")
                    nc.scalar.dma_start(x8_sb[:], x8_d[0, :, j])
                    x8_t[j] = x8_sb
                xl8_t = {}
                for j in range(4):
                    xl8_sb = xcp.tile([128, KO, 128], F8, tag="xl8")
                    nc.gpsimd.dma_start(xl8_sb[:], xl8_d[0, :, j])
                    xl8_t[j] = xl8_sb
                for j in range(4):
                    xss[0][j] = (xh_t[j], x8_t[j], xl8_t[j])

                dests = {}
                ctxs = {}
                dests[0], ctx0 = make_ctx(0)
                ctxs[0] = ctx0
                # block 0: V tiles first (v16 lands ~5x sooner than w16+w8+wl8)
                for j in range(4):
                    for half in range(2):
                        v_tile(xss[0][j], half, dests[0][2], j, evac_dve=True)
                for win in ((0, 1), (2, 3)):
                    for j in range(4):
                        for n in win:
                            qk_tile(xss[0][j], n,
                                    dests[0][0] if n < 2 else dests[0][1], j)

                for blk in range(1, NBLK):
                    xss[blk] = {j: load_chunk(blk, j) for j in range(4)}
                    dests[blk], ctxs[blk] = make_ctx(blk)
                    Qb, Kb, Vb = dests[blk]
                    if blk < NBLK - 2:
                        projections(blk, dests[blk], xss[blk])
                        attention(blk - 1, ctxs[blk - 1], slice(0, HEADS))
                    elif blk == NBLK - 2:
                        # block 2 also gets window treatment so its scores
                        # (a long DVE-only reduce chain feeding s) start as
                        # early as possible
                        for j in range(4):
                            for n in (0, 2):
                                qk_tile(xss[blk][j], n, Qb if n < 2 else Kb, j)
                        attention(blk - 1, ctxs[blk - 1], slice(0, HEADS))
                        for j in range(4):
                            for n in (1, 3):
                                qk_tile(xss[blk][j], n, Qb if n < 2 else Kb, j)
                        for j in range(4):
                            for half in range(2):
                                v_tile(xss[blk][j], half, Vb, j)
                    else:
                        # block 3: head-half windows + interleaved emission.
                        # att2's scores are interleaved with block 3's QK
                        # windows per head-half; att2's+att3's mixes and
                        # transposes go to the post-collective bundle -- only
                        # scores/c feed the s path.
                        for j in range(4):
                            for n in (0, 2):
                                qk_tile(xss[blk][j], n, Qb if n < 2 else Kb, j)
                        scores_part(ctxs[blk - 1], slice(0, 6))
                        nc.sync.dma_start(wvT_sb[:], wvT_d[:])
                        nc.sync.dma_start(wo_sb[:], wo_d[:])
                        for j in range(4):
                            for n in (1, 3):
                                qk_tile(xss[blk][j], n, Qb if n < 2 else Kb, j)
                        scores_part(ctxs[blk - 1], slice(6, HEADS))
                        cx_part(ctxs[blk - 1])
                        scores_part(ctxs[blk], slice(0, 6))
                        for j in range(4):
                            for half in range(2):
                                v_tile(xss[blk][j], half, Vb, j)
                        # hh1 scores BEFORE the mixes: c (-> s -> collective)
                        # is the critical path, the mix only feeds phase B
                        scores_part(ctxs[blk], slice(6, HEADS))
                        cx_part(ctxs[blk])

                # ---------- s = cx @ Wv per head (72 1-row matmuls) ------
                sp6 = spp.tile([128, KO], F32, tag="sp")
                for h in range(HEADS):
                    koc, off = divmod(h, 2)
                    out = sp6[off * 64:(off + 1) * 64, koc:koc + 1]
                    for ko in range(KO):
                        nc.tensor.matmul(
                            out, lhsT=wvT_sb[:, ko, h * HD:(h + 1) * HD],
                            rhs=cx_acc[:, ko, h:h + 1],
                            start=(ko == 0), stop=(ko == KO - 1),
                        )

                nc.scalar.copy(s_acc[:], sp6[:])

                # ---------- AllGather of per-core s ----------------------
                cc_in = dramp.tile([128, KO], F32)
                cc_out = dramp.tile([4, 128, KO], F32)
                nc.scalar.dma_start(cc_in[:], s_acc[:])
                nc.gpsimd.collective_compute(
                    "AllGather",
                    mybir.AluOpType.bypass,
                    replica_groups=[[0, 1, 2, 3], [4, 5, 6, 7]],
                    ins=[cc_in[:].opt()],
                    outs=[cc_out[:].opt()],
                )
                # PE pstate warm-up: throwaway matmuls emitted first so they
                # run right at the collective's exit and phase B starts at
                # full clock instead of ramping through the slow p-states
                wps = mmp.tile([128, 384], F32, tag="mm")
                for i in range(9):
                    nc.tensor.matmul(
                        wps[:], lhsT=oT_tiles[(0, 0)][:, 0, :],
                        rhs=w16_sb[:, 0, 0:384],
                        start=True, stop=True,
                    )

                sg = constp.tile([128, KO, 4], F32)
                nc.gpsimd.dma_start(
                    sg[:], cc_out[:].rearrange("r p k -> p k r")
                )
                s_tot = constp.tile([128, KO], F32)
                nc.vector.reduce_sum(s_tot[:], sg[:], axis=AX.X)
                r_sb = constp.tile([128, KO], F32)
                nc.vector.reciprocal(r_sb[:], s_tot[:])

                # r-fold on ACT in place (wo_sb becomes ws), emitted BEFORE
                # the late mixes so phase B's matmuls start right after the
                # reciprocal
                for ko in range(KO):
                    nc.scalar.activation(
                        wo_sb[:, ko, :], wo_sb[:, ko, :], ACTF.Copy,
                        scale=r_sb[:, ko:ko + 1],
                    )

                # blocks 2+3 mix + transposes: emitted after the collective
                # (so its engine-drain entry gate doesn't wait on them);
                # they run during the phase-B lead-in and are only needed
                # for the last half of phase B
                # muls all on Pool here: DVE must be free for the
                # s_tot reduce + reciprocal that gate phase B
                mix_part(ctxs[NBLK - 2], slice(0, HEADS))
                transpose_part(NBLK - 2, ctxs[NBLK - 2])
                mix_part(ctxs[NBLK - 1], slice(0, 6))
                mix_part(ctxs[NBLK - 1], slice(6, HEADS))
                transpose_part(NBLK - 1, ctxs[NBLK - 1])

            # =============== Phase B: out-projection =====================
            with tc.tile_pool(name="fin", bufs=2) as finp:
                ws_sb = wo_sb   # folded in place above
                for blk in range(NBLK):
                    ystage = finp.tile([128, 4, EMBED], F32, tag="ys")
                    for j in range(4):
                        oT = oT_tiles[(blk, j)]
                        for half in range(2):
                            pf = mmp.tile([128, 384], F32, tag="mm")
                            for ko in range(KO):
                                nc.tensor.matmul(
                                    pf[:],
                                    lhsT=oT[:, ko, :],
                                    rhs=ws_sb[:, ko, half * 384:(half + 1) * 384],
                                    start=(ko == 0),
                                    stop=(ko == KO - 1),
                                )
                            nc.scalar.copy(
                                ystage[:, j, half * 384:(half + 1) * 384], pf[:]
                            )
                        # per-(blk, j) DMA keeps the y tail short; the very
                        # last chunk goes per-half so its DMA tail is shorter
                        rows = blk * 512 + j * 128
                        if blk == NBLK - 1 and j == 3:
                            # ACT-issued (same queue as the evacuations:
                            # no cross-engine sem hop on the final tail)
                            for half in range(2):
                                nc.scalar.dma_start(
                                    y_d[rows:rows + 128,
                                        half * 384:(half + 1) * 384],
                                    ystage[:, j, half * 384:(half + 1) * 384],
                                )
                        else:
                            nc.sync.dma_start(
                                y_d[rows:rows + 128, :], ystage[:, j, :]
                            )

    nc.finalize()
    return nc


def _host_shard(x, Wqkv, Wout):
    """Build per-core input maps."""
    import ml_dtypes

    E4 = ml_dtypes.float8_e4m3
    x = np.ascontiguousarray(np.asarray(x, dtype=np.float32))
    Wqkv = np.asarray(Wqkv, dtype=np.float32)
    Wout = np.asarray(Wout, dtype=np.float32)

    wq = np.ascontiguousarray(
        Wqkv.T.reshape(KO, 128, 3 * EMBED).transpose(1, 0, 2)
    )
    wqk = wq[:, :, : 2 * EMBED]
    w16 = np.ascontiguousarray((wqk * S15).astype(np.float16))
    wl = wqk - w16.astype(np.float32) / np.float32(S15)
    w8 = np.ascontiguousarray((wqk * 16.0).astype(E4))
    wl8 = np.ascontiguousarray((wl * S15).astype(E4))
    v16 = np.ascontiguousarray((wq[:, :, 2 * EMBED:] * S15).astype(np.float16))
    wvT = np.ascontiguousarray(
        Wqkv[2 * EMBED:].T.reshape(KO, 128, EMBED).transpose(1, 0, 2)
    )
    wo = np.ascontiguousarray(
        Wout.T.reshape(KO, 128, EMBED).transpose(1, 0, 2).astype(np.float16)
    )
    m2 = (np.arange(128) % 2 == 0).astype(np.float32).reshape(128, 1)
    m4 = (np.arange(128) % 4 == 0).astype(np.float32).reshape(128, 1)

    in_maps = []
    for c in range(NCORES):
        bc, q = divmod(c, 4)
        xb = x[bc].reshape(4, 4, 4, 128, EMBED)  # [j, q, blk, g, e]
        mine = xb[:, q]                          # [j, blk, g, e]
        t = np.ascontiguousarray(mine.transpose(1, 0, 2, 3))  # [blk, j, g, e]
        # col-major: [blk, k, j, ko, g]
        tc_ = t.reshape(NBLK, 4, 128, KO, 128).transpose(0, 4, 1, 3, 2)
        xch = np.ascontiguousarray(tc_.astype(np.float16))
        xl = tc_ - xch.astype(np.float32)
        x8 = np.ascontiguousarray(tc_.astype(E4))
        xl8 = np.ascontiguousarray((xl * 2048.0).astype(E4))
        # row-major fp32: [blk, g, j, e]
        xr = np.ascontiguousarray(t.transpose(0, 2, 1, 3))
        in_maps.append(
            {
                "xch": xch, "x8": x8, "xl8": xl8, "xr": xr,
                "w16": w16, "w8": w8, "wl8": wl8,
                "v16": v16, "wvT": wvT,
                "wo": wo, "m2": m2, "m4": m4,
            }
        )
    return in_maps


def _host_assemble(results):
    y = np.empty((B, N, EMBED), dtype=np.float32)
    for c in range(NCORES):
        bc, q = divmod(c, 4)
        yc = np.asarray(results[c]["y"])  # [2048, 768], rows (blk, j, g)
        part = yc.reshape(4, 4, 128, EMBED).transpose(1, 0, 2, 3)  # [j, blk, g, e]
        y[bc].reshape(4, 4, 4, 128, EMBED)[:, q] = part
    return y


def kernel(x, Wqkv, Wout):
    from concourse.bass_utils import run_bass_kernel_spmd

    if "nc" not in _COMPILED:
        _COMPILED["nc"] = _build_program()
    nc = _COMPILED["nc"]

    in_maps = _host_shard(x, Wqkv, Wout)
    res = run_bass_kernel_spmd(nc, in_maps, core_ids=list(range(NCORES)))
    _COMPILED["last_result"] = res
    return _host_assemble(res.results)


if __name__ == "__main__":
    # smoke build
    nc = _build_program()
    print("built ok; instructions:", len(nc.inst_map))
